# revision 1
# baseline (speedup 1.0000x reference)
"""GraphSAGE 2-layer forward on 8 Trainium2 NeuronCores.

Strategy (sharding_hint: partition edges by destination node):
  - Nodes are padded to NP=50176 = 8 cores * 49 chunks * 128 and sharded by
    destination across the 8 cores (6272 dst nodes per core).
  - Edges are bucketed by dst chunk (128 dst nodes per chunk).  Within a
    bucket, edges are split into src<32768 ("lo") and src>=32768 ("hi")
    sub-lists because dma_gather indices are int16.
  - Per chunk, messages x[src] are fetched with dma_gather (one instruction
    per lo/hi block), and aggregated with one-hot matmuls on the tensor
    engine: for each tile of 128 edges, a [128 edges, 128 dst] selection
    matrix oh[e,d] = (dstrel[e]==d) * (1/deg(dst[e])) is built on the vector
    engine and matmul-accumulated into PSUM.  The per-edge weight bakes the
    mean division into the aggregation.
  - Layer 2's aggregated term uses p = h @ W2_l (64-dim) so only 64 floats
    per edge are gathered; p is exchanged between cores with an AllGather.
  - All structure (bucket tile counts) is made uniform across cores by
    padding each chunk-slot to the max tile count over the 8 cores; padded
    slots gather row 0 with weight 0 so they contribute nothing.
"""

import sys

sys.path.insert(0, "/opt/trn_rl_repo")

import numpy as np

N = 50000
E = 800000
D_IN, D_HID, D_OUT = 128, 128, 64
N_CORES = 8
CHUNK = 128
C_PER_CORE = 49
NODES_PC = C_PER_CORE * CHUNK  # 6272
NP_ = N_CORES * NODES_PC  # 50176
NCH = N_CORES * C_PER_CORE  # 392 chunks
SPLIT = 32768


def _preprocess(x, edge_index):
    """Host-side index preprocessing. Returns per-core input maps + profile."""
    x = np.asarray(x, dtype=np.float32)
    src = np.asarray(edge_index[0], dtype=np.int64)
    dst = np.asarray(edge_index[1], dtype=np.int64)

    cnt = np.bincount(dst, minlength=NP_).astype(np.float32)
    inv = (1.0 / np.maximum(cnt, 1.0)).astype(np.float32)
    w_edge = inv[dst]

    chunk = dst // CHUNK
    hi_flag = (src >= SPLIT).astype(np.int64)
    order = np.lexsort((hi_flag, chunk))
    s_src = src[order]
    s_dst = dst[order]
    s_w = w_edge[order]

    lo_mask = src < SPLIT
    n_lo = np.bincount(chunk[lo_mask], minlength=NCH)
    n_hi = np.bincount(chunk[~lo_mask], minlength=NCH)
    n_tot = n_lo + n_hi
    start = np.zeros(NCH, np.int64)
    start[1:] = np.cumsum(n_tot)[:-1]

    t_lo_c = -(-n_lo // 128)  # ceil
    t_hi_c = -(-n_hi // 128)
    # slot profile: max tile count over the 8 cores for each of 49 slots
    TL = t_lo_c.reshape(N_CORES, C_PER_CORE).max(axis=0)
    TH = t_hi_c.reshape(N_CORES, C_PER_CORE).max(axis=0)
    TT = TL + TH
    T_total = int(TT.sum())
    S = T_total * 128  # total gather slots per core per layer

    x_pad = np.zeros((NP_, D_IN), np.float32)
    x_pad[:N] = x

    per_core = []
    for k in range(N_CORES):
        idx16 = np.zeros(S, np.int16)
        drel = np.full(S, 200.0, np.float32)
        warr = np.zeros(S, np.float32)
        pos = 0
        for j in range(C_PER_CORE):
            c = k * C_PER_CORE + j
            nl, nh = int(n_lo[c]), int(n_hi[c])
            s0 = int(start[c])
            # lo block
            idx16[pos : pos + nl] = s_src[s0 : s0 + nl]
            drel[pos : pos + nl] = s_dst[s0 : s0 + nl] % CHUNK
            warr[pos : pos + nl] = s_w[s0 : s0 + nl]
            pos += int(TL[j]) * 128
            # hi block
            idx16[pos : pos + nh] = s_src[s0 + nl : s0 + nl + nh] - SPLIT
            drel[pos : pos + nh] = s_dst[s0 + nl : s0 + nl + nh] % CHUNK
            warr[pos : pos + nh] = s_w[s0 + nl : s0 + nl + nh]
            pos += int(TH[j]) * 128
        assert pos == S
        idx_wrapped = np.ascontiguousarray(
            np.tile(idx16.reshape(S // 16, 16).T, (8, 1))
        )  # [128, S/16]
        drel2 = np.ascontiguousarray(drel.reshape(T_total, 128).T)  # [128, T]
        warr2 = np.ascontiguousarray(warr.reshape(T_total, 128).T)
        xT_k = np.ascontiguousarray(x_pad[k * NODES_PC : (k + 1) * NODES_PC].T)
        per_core.append(
            {
                "x_g": x_pad,
                "xT": xT_k,
                "idx": idx_wrapped,
                "drel": drel2,
                "wgt": warr2,
            }
        )
    return per_core, [int(v) for v in TL], [int(v) for v in TH]


def _build(TL, TH, stage=3, n_chunks=C_PER_CORE):
    """stage 1: layer-1 only (out = p per-core); 2: + allgather; 3: full."""
    import concourse.bacc as bacc
    import concourse.mybir as mybir
    from concourse.tile import TileContext

    f32 = mybir.dt.float32
    i16 = mybir.dt.int16
    TT = [a + b for a, b in zip(TL, TH)]
    T_total = sum(TT)
    S16 = T_total * 8  # idx table columns (16 idx per column)
    TTmax = max(TT)

    nc = bacc.Bacc(
        "TRN2",
        target_bir_lowering=False,
        debug=False,
        enable_asserts=False,
        num_devices=N_CORES,
    )

    x_g = nc.dram_tensor("x_g", [NP_, D_IN], f32, kind="ExternalInput").ap()
    xT_d = nc.dram_tensor("xT", [128, NODES_PC], f32, kind="ExternalInput").ap()
    idx_d = nc.dram_tensor("idx", [128, S16], i16, kind="ExternalInput").ap()
    drel_d = nc.dram_tensor("drel", [128, T_total], f32, kind="ExternalInput").ap()
    wgt_d = nc.dram_tensor("wgt", [128, T_total], f32, kind="ExternalInput").ap()
    w1l_d = nc.dram_tensor("W1_l", [D_IN, D_HID], f32, kind="ExternalInput").ap()
    w1r_d = nc.dram_tensor("W1_r", [D_IN, D_HID], f32, kind="ExternalInput").ap()
    w2l_d = nc.dram_tensor("W2_l", [D_HID, D_OUT], f32, kind="ExternalInput").ap()
    w2r_d = nc.dram_tensor("W2_r", [D_HID, D_OUT], f32, kind="ExternalInput").ap()
    b1_d = nc.dram_tensor("b1", [D_HID, 1], f32, kind="ExternalInput").ap()
    b2_d = nc.dram_tensor("b2", [1, D_OUT], f32, kind="ExternalInput").ap()
    out_d = nc.dram_tensor("out", [NODES_PC, D_OUT], f32, kind="ExternalOutput").ap()
    p_full = nc.dram_tensor(
        "p_full", [NP_, D_OUT], f32, kind="Internal", addr_space="Shared"
    ).ap()

    relu = mybir.ActivationFunctionType.Relu
    is_eq = mybir.AluOpType.is_equal
    mult = mybir.AluOpType.mult

    with TileContext(nc) as tc:
        with (
            tc.tile_pool(name="persist", bufs=1) as pp,
            tc.tile_pool(name="dram", bufs=1, space="DRAM") as dp,
            tc.tile_pool(name="msg", bufs=2) as mpool,
            tc.tile_pool(name="oh", bufs=4) as ohpool,
            tc.tile_pool(name="stage", bufs=3) as spool,
            tc.tile_pool(name="psA", bufs=2, space="PSUM") as psA,
            tc.tile_pool(name="psH", bufs=2, space="PSUM") as psH,
            tc.tile_pool(name="psO", bufs=2, space="PSUM") as psO,
        ):
            xT_sb = pp.tile([128, NODES_PC], f32)
            nc.sync.dma_start(out=xT_sb[:], in_=xT_d)
            idx_sb = pp.tile([128, S16], i16)
            nc.sync.dma_start(out=idx_sb[:], in_=idx_d)
            drel_sb = pp.tile([128, T_total], f32)
            nc.sync.dma_start(out=drel_sb[:], in_=drel_d)
            wgt_sb = pp.tile([128, T_total], f32)
            nc.sync.dma_start(out=wgt_sb[:], in_=wgt_d)
            w1l_sb = pp.tile([D_IN, D_HID], f32)
            nc.sync.dma_start(out=w1l_sb[:], in_=w1l_d)
            w1r_sb = pp.tile([D_IN, D_HID], f32)
            nc.sync.dma_start(out=w1r_sb[:], in_=w1r_d)
            w2l_sb = pp.tile([D_HID, D_OUT], f32)
            nc.sync.dma_start(out=w2l_sb[:], in_=w2l_d)
            w2r_sb = pp.tile([D_HID, D_OUT], f32)
            nc.sync.dma_start(out=w2r_sb[:], in_=w2r_d)
            b1_sb = pp.tile([D_HID, 1], f32)
            nc.sync.dma_start(out=b1_sb[:], in_=b1_d)
            b2_sb = pp.tile([1, D_OUT], f32)
            nc.sync.dma_start(out=b2_sb[:], in_=b2_d)
            iota_sb = pp.tile([128, 128], f32)
            nc.gpsimd.iota(
                iota_sb[:],
                pattern=[[1, 128]],
                base=0,
                channel_multiplier=0,
                allow_small_or_imprecise_dtypes=True,
            )
            ones_sb = pp.tile([1, 128], f32)
            nc.vector.memset(ones_sb[:], 1.0)
            h_all = pp.tile([128, NODES_PC], f32)
            p_bounce = dp.tile([NODES_PC, D_OUT], f32)

            # ---------------- phase A: layer 1 + p = h @ W2_l ----------------
            tb = 0
            for j in range(n_chunks):
                tl, th, tt = TL[j], TH[j], TT[j]
                msg = mpool.tile([128, TTmax * 128], f32, tag="msg")
                if tl:
                    nc.gpsimd.dma_gather(
                        out_ap=msg[:, : tl * 128].rearrange("p (t e) -> p t e", e=128),
                        in_ap=x_g[0:SPLIT, :],
                        idxs_ap=idx_sb[:, tb * 8 : (tb + tl) * 8],
                        num_idxs=tl * 128,
                        num_idxs_reg=tl * 128,
                        elem_size=128,
                        single_packet=False,
                    )
                if th:
                    nc.gpsimd.dma_gather(
                        out_ap=msg[:, tl * 128 : tt * 128].rearrange(
                            "p (t e) -> p t e", e=128
                        ),
                        in_ap=x_g[SPLIT:NP_, :],
                        idxs_ap=idx_sb[:, (tb + tl) * 8 : (tb + tt) * 8],
                        num_idxs=th * 128,
                        num_idxs_reg=th * 128,
                        elem_size=128,
                        single_packet=False,
                    )
                pa = psA.tile([128, 128], f32, tag="agg")
                for t in range(tt):
                    oh = ohpool.tile([128, 128], f32, tag="oh")
                    nc.vector.tensor_scalar(
                        out=oh[:],
                        in0=iota_sb[:],
                        scalar1=drel_sb[:, tb + t : tb + t + 1],
                        scalar2=wgt_sb[:, tb + t : tb + t + 1],
                        op0=is_eq,
                        op1=mult,
                    )
                    nc.tensor.matmul(
                        out=pa[:],
                        lhsT=msg[:, t * 128 : (t + 1) * 128],
                        rhs=oh[:],
                        start=(t == 0),
                        stop=(t == tt - 1),
                    )
                # meanT[in, dst] (weights baked in => already the mean)
                meanT = spool.tile([128, 128], f32, tag="meanT")
                nc.scalar.copy(out=meanT[:], in_=pa[:])
                jsl = slice(j * 128, (j + 1) * 128)
                ph = psH.tile([128, 128], f32, tag="h")
                nc.tensor.matmul(
                    out=ph[:], lhsT=w1l_sb[:], rhs=meanT[:], start=True, stop=False
                )
                nc.tensor.matmul(
                    out=ph[:], lhsT=w1r_sb[:], rhs=xT_sb[:, jsl], start=False, stop=True
                )
                nc.scalar.activation(
                    out=h_all[:, jsl], in_=ph[:], func=relu, bias=b1_sb[:, 0:1], scale=1.0
                )
                po = psO.tile([128, D_OUT], f32, tag="p")
                nc.tensor.matmul(
                    out=po[:], lhsT=h_all[:, jsl], rhs=w2l_sb[:], start=True, stop=True
                )
                p_sb = spool.tile([128, D_OUT], f32, tag="p_sb")
                nc.scalar.copy(out=p_sb[:], in_=po[:])
                nc.sync.dma_start(out=p_bounce[jsl, :], in_=p_sb[:])
                if stage == 1:
                    nc.sync.dma_start(out=out_d[jsl, :], in_=p_sb[:])
                tb += tt

            # ---------------- all-gather p ----------------
            if stage >= 2:
                nc.gpsimd.collective_compute(
                    "AllGather",
                    mybir.AluOpType.bypass,
                    replica_groups=[list(range(N_CORES))],
                    ins=[p_bounce[:]],
                    outs=[p_full],
                )

            if stage == 2:
                for j in range(n_chunks):
                    jsl = slice(j * 128, (j + 1) * 128)
                    st = spool.tile([128, D_OUT], f32, tag="out_sb")
                    nc.sync.dma_start(out=st[:], in_=p_full[jsl, :])
                    nc.sync.dma_start(out=out_d[jsl, :], in_=st[:])

            # ---------------- phase B: layer 2 ----------------
            tb = 0
            for j in range(n_chunks if stage >= 3 else 0):
                tl, th, tt = TL[j], TH[j], TT[j]
                msg2 = mpool.tile([128, TTmax * D_OUT], f32, tag="msg2")
                if tl:
                    nc.gpsimd.dma_gather(
                        out_ap=msg2[:, : tl * D_OUT].rearrange(
                            "p (t e) -> p t e", e=D_OUT
                        ),
                        in_ap=p_full[0:SPLIT, :],
                        idxs_ap=idx_sb[:, tb * 8 : (tb + tl) * 8],
                        num_idxs=tl * 128,
                        num_idxs_reg=tl * 128,
                        elem_size=D_OUT,
                        single_packet=False,
                    )
                if th:
                    nc.gpsimd.dma_gather(
                        out_ap=msg2[:, tl * D_OUT : tt * D_OUT].rearrange(
                            "p (t e) -> p t e", e=D_OUT
                        ),
                        in_ap=p_full[SPLIT:NP_, :],
                        idxs_ap=idx_sb[:, (tb + tl) * 8 : (tb + tt) * 8],
                        num_idxs=th * 128,
                        num_idxs_reg=th * 128,
                        elem_size=D_OUT,
                        single_packet=False,
                    )
                jsl = slice(j * 128, (j + 1) * 128)
                pf = psA.tile([128, D_OUT], f32, tag="fin")
                for t in range(tt):
                    oh = ohpool.tile([128, 128], f32, tag="oh")
                    nc.vector.tensor_scalar(
                        out=oh[:],
                        in0=iota_sb[:],
                        scalar1=drel_sb[:, tb + t : tb + t + 1],
                        scalar2=wgt_sb[:, tb + t : tb + t + 1],
                        op0=is_eq,
                        op1=mult,
                    )
                    nc.tensor.matmul(
                        out=pf[:],
                        lhsT=oh[:],
                        rhs=msg2[:, t * D_OUT : (t + 1) * D_OUT],
                        start=(t == 0),
                        stop=False,
                    )
                nc.tensor.matmul(
                    out=pf[:], lhsT=h_all[:, jsl], rhs=w2r_sb[:], start=False, stop=False
                )
                nc.tensor.matmul(
                    out=pf[:], lhsT=ones_sb[:], rhs=b2_sb[:], start=False, stop=True
                )
                out_sb = spool.tile([128, D_OUT], f32, tag="out_sb")
                nc.scalar.copy(out=out_sb[:], in_=pf[:])
                nc.sync.dma_start(out=out_d[jsl, :], in_=out_sb[:])
                tb += tt

    nc.compile()
    return nc


def kernel(
    x,
    edge_index,
    W1_l,
    b1,
    W1_r,
    W2_l,
    b2,
    W2_r,
):
    from concourse.bass_utils import run_bass_kernel_spmd

    per_core, TL, TH = _preprocess(x, edge_index)
    nc = _build(TL, TH)

    shared = {
        "W1_l": np.ascontiguousarray(np.asarray(W1_l, np.float32)),
        "W1_r": np.ascontiguousarray(np.asarray(W1_r, np.float32)),
        "W2_l": np.ascontiguousarray(np.asarray(W2_l, np.float32)),
        "W2_r": np.ascontiguousarray(np.asarray(W2_r, np.float32)),
        "b1": np.ascontiguousarray(np.asarray(b1, np.float32).reshape(D_HID, 1)),
        "b2": np.ascontiguousarray(np.asarray(b2, np.float32).reshape(1, D_OUT)),
    }
    in_maps = [{**pc, **shared} for pc in per_core]

    res = run_bass_kernel_spmd(nc, in_maps, core_ids=list(range(N_CORES)))
    out = np.concatenate([r["out"] for r in res.results], axis=0)
    return out[:N].astype(np.float32)


if __name__ == "__main__":
    rng = np.random.default_rng(0)
    x = rng.standard_normal((N, D_IN), dtype=np.float32)
    ei = rng.integers(0, N, size=(2, E), dtype=np.int64)
    s = 1.0 / np.sqrt(D_IN)
    w1l = rng.uniform(-s, s, (D_IN, D_HID)).astype(np.float32)
    w1r = rng.uniform(-s, s, (D_IN, D_HID)).astype(np.float32)
    s2 = 1.0 / np.sqrt(D_HID)
    w2l = rng.uniform(-s2, s2, (D_HID, D_OUT)).astype(np.float32)
    w2r = rng.uniform(-s2, s2, (D_HID, D_OUT)).astype(np.float32)
    out = kernel(
        x=x,
        edge_index=ei,
        W1_l=w1l,
        b1=np.zeros(D_HID, np.float32),
        W1_r=w1r,
        W2_l=w2l,
        b2=np.zeros(D_OUT, np.float32),
        W2_r=w2r,
    )
    print(out.shape, out.dtype)



# revision 4
# speedup vs baseline: 1.1325x; 1.1325x over previous
"""GraphSAGE 2-layer forward on 8 Trainium2 NeuronCores.

Strategy (sharding_hint: partition edges by destination node):
  - Nodes are padded to NP=50176 = 8 cores * 49 chunks * 128 and sharded by
    destination across the 8 cores (6272 dst nodes per core).
  - Edges are bucketed by dst chunk (128 dst nodes per chunk).  Within a
    bucket, edges are split into src<32768 ("lo") and src>=32768 ("hi")
    sub-lists because dma_gather indices are int16.
  - Per chunk, messages x[src] are fetched with dma_gather (one instruction
    per lo/hi block) in bf16 (256B descriptors), and aggregated with one-hot
    matmuls on the tensor engine.  One-hot (0/1) selection matrices for a
    whole chunk (up to TTmax tiles) are built in a single batched DVE
    tensor_tensor is_equal op against a broadcast drel table; the 1/deg mean
    scaling is applied after aggregation (free-dim inv_rep multiply for
    layer 1, per-partition scalar multiply for layer 2).
  - Layer 2 gathers h @ W2_l (= p, 64 wide) stored padded to 128 bf16 lanes
    so the same int16 index tables are reused; p is exchanged between cores
    with an AllGather.
  - All compute matmuls run in bf16 (messages, one-hots, weights) with fp32
    PSUM accumulation.
"""

import sys

sys.path.insert(0, "/opt/trn_rl_repo")

import numpy as np

N = 50000
E = 800000
D_IN, D_HID, D_OUT = 128, 128, 64
N_CORES = 8
CHUNK = 128
C_PER_CORE = 49
NODES_PC = C_PER_CORE * CHUNK  # 6272
NP_ = N_CORES * NODES_PC  # 50176
NCH = N_CORES * C_PER_CORE  # 392 chunks
SPLIT = 32768


def _preprocess(x, edge_index):
    """Host-side index preprocessing. Returns per-core input maps + profile."""
    import ml_dtypes

    x = np.asarray(x, dtype=np.float32)
    src = np.asarray(edge_index[0], dtype=np.int64)
    dst = np.asarray(edge_index[1], dtype=np.int64)

    cnt = np.bincount(dst, minlength=NP_).astype(np.float32)
    inv = (1.0 / np.maximum(cnt, 1.0)).astype(np.float32)

    chunk = dst // CHUNK
    hi_flag = (src >= SPLIT).astype(np.int64)
    order = np.lexsort((hi_flag, chunk))
    s_src = src[order]
    s_dst = dst[order]

    lo_mask = src < SPLIT
    n_lo = np.bincount(chunk[lo_mask], minlength=NCH)
    n_hi = np.bincount(chunk[~lo_mask], minlength=NCH)
    n_tot = n_lo + n_hi
    start = np.zeros(NCH, np.int64)
    start[1:] = np.cumsum(n_tot)[:-1]

    t_lo_c = -(-n_lo // 128)  # ceil
    t_hi_c = -(-n_hi // 128)
    # slot profile: max tile count over the 8 cores for each of 49 slots
    TL = t_lo_c.reshape(N_CORES, C_PER_CORE).max(axis=0)
    TH = t_hi_c.reshape(N_CORES, C_PER_CORE).max(axis=0)
    TT = TL + TH
    T_total = int(TT.sum())
    S = T_total * 128  # total gather slots per core per layer

    x_pad = np.zeros((NP_, D_IN), np.float32)
    x_pad[:N] = x
    x_g16 = x_pad.astype(ml_dtypes.bfloat16)

    # inv_rep: [128, NODES_PC] per core, row-replicated inverse degree (bf16)
    # inv_colT: [128, C_PER_CORE] per core, per-dst-lane inverse degree (f32)
    per_core = []
    for k in range(N_CORES):
        idx16 = np.zeros(S, np.int16)
        drel = np.full(S, 200.0, np.float32)
        pos = 0
        for j in range(C_PER_CORE):
            c = k * C_PER_CORE + j
            nl, nh = int(n_lo[c]), int(n_hi[c])
            s0 = int(start[c])
            # lo block
            idx16[pos : pos + nl] = s_src[s0 : s0 + nl]
            drel[pos : pos + nl] = s_dst[s0 : s0 + nl] % CHUNK
            pos += int(TL[j]) * 128
            # hi block
            idx16[pos : pos + nh] = s_src[s0 + nl : s0 + nl + nh] - SPLIT
            drel[pos : pos + nh] = s_dst[s0 + nl : s0 + nl + nh] % CHUNK
            pos += int(TH[j]) * 128
        assert pos == S
        idx_wrapped = np.ascontiguousarray(
            np.tile(idx16.reshape(S // 16, 16).T, (8, 1))
        )  # [128, S/16]
        drel2 = np.ascontiguousarray(
            drel.reshape(T_total, 128).T.astype(ml_dtypes.bfloat16)
        )  # [128, T]
        inv_k = inv[k * NODES_PC : (k + 1) * NODES_PC]
        inv_rep = np.ascontiguousarray(
            np.tile(inv_k[None, :], (128, 1)).astype(ml_dtypes.bfloat16)
        )  # [128, NODES_PC]
        inv_colT = np.ascontiguousarray(
            inv_k.reshape(C_PER_CORE, 128).T.astype(np.float32)
        )  # [128, 49]
        xT_k = np.ascontiguousarray(
            x_pad[k * NODES_PC : (k + 1) * NODES_PC].T.astype(ml_dtypes.bfloat16)
        )
        per_core.append(
            {
                "x_g": x_g16,
                "xT": xT_k,
                "idx": idx_wrapped,
                "drel": drel2,
                "inv_rep": inv_rep,
                "inv_colT": inv_colT,
            }
        )
    return per_core, [int(v) for v in TL], [int(v) for v in TH]


def _shared_inputs(W1_l, b1, W1_r, W2_l, b2, W2_r):
    import ml_dtypes

    bf = ml_dtypes.bfloat16
    return {
        "W1_l": np.ascontiguousarray(np.asarray(W1_l, np.float32).astype(bf)),
        "W1_r": np.ascontiguousarray(np.asarray(W1_r, np.float32).astype(bf)),
        "W2_l": np.ascontiguousarray(np.asarray(W2_l, np.float32).astype(bf)),
        "W2_r": np.ascontiguousarray(np.asarray(W2_r, np.float32).astype(bf)),
        "b1": np.ascontiguousarray(np.asarray(b1, np.float32).reshape(D_HID, 1)),
        "b2": np.ascontiguousarray(
            np.asarray(b2, np.float32).astype(bf).reshape(1, D_OUT)
        ),
    }


def _build(TL, TH, n_chunks=C_PER_CORE):
    import concourse.bacc as bacc
    import concourse.mybir as mybir
    from concourse.tile import TileContext

    f32 = mybir.dt.float32
    bf16 = mybir.dt.bfloat16
    i16 = mybir.dt.int16
    TT = [a + b for a, b in zip(TL, TH)]
    T_total = sum(TT)
    S16 = T_total * 8  # idx table columns (16 idx per column)
    TTmax = max(TT)

    nc = bacc.Bacc(
        "TRN2",
        target_bir_lowering=False,
        debug=False,
        enable_asserts=False,
        num_devices=N_CORES,
    )

    x_g = nc.dram_tensor("x_g", [NP_, D_IN], bf16, kind="ExternalInput").ap()
    xT_d = nc.dram_tensor("xT", [128, NODES_PC], bf16, kind="ExternalInput").ap()
    idx_d = nc.dram_tensor("idx", [128, S16], i16, kind="ExternalInput").ap()
    drel_d = nc.dram_tensor("drel", [128, T_total], bf16, kind="ExternalInput").ap()
    invr_d = nc.dram_tensor(
        "inv_rep", [128, NODES_PC], bf16, kind="ExternalInput"
    ).ap()
    invc_d = nc.dram_tensor(
        "inv_colT", [128, C_PER_CORE], f32, kind="ExternalInput"
    ).ap()
    w1l_d = nc.dram_tensor("W1_l", [D_IN, D_HID], bf16, kind="ExternalInput").ap()
    w1r_d = nc.dram_tensor("W1_r", [D_IN, D_HID], bf16, kind="ExternalInput").ap()
    w2l_d = nc.dram_tensor("W2_l", [D_HID, D_OUT], bf16, kind="ExternalInput").ap()
    w2r_d = nc.dram_tensor("W2_r", [D_HID, D_OUT], bf16, kind="ExternalInput").ap()
    b1_d = nc.dram_tensor("b1", [D_HID, 1], f32, kind="ExternalInput").ap()
    b2_d = nc.dram_tensor("b2", [1, D_OUT], bf16, kind="ExternalInput").ap()
    out_d = nc.dram_tensor("out", [NODES_PC, D_OUT], f32, kind="ExternalOutput").ap()
    p_full = nc.dram_tensor(
        "p_full", [NP_, 128], bf16, kind="Internal", addr_space="Shared"
    ).ap()

    relu = mybir.ActivationFunctionType.Relu
    is_eq = mybir.AluOpType.is_equal
    mult = mybir.AluOpType.mult
    add = mybir.AluOpType.add

    with TileContext(nc) as tc:
        with (
            tc.tile_pool(name="persist", bufs=1) as pp,
            tc.tile_pool(name="dram", bufs=1, space="DRAM") as dp,
            tc.tile_pool(name="msg", bufs=2) as mpool,
            tc.tile_pool(name="oh", bufs=3) as ohpool,
            tc.tile_pool(name="stage", bufs=3) as spool,
            tc.tile_pool(name="psA", bufs=2, space="PSUM") as psA,
            tc.tile_pool(name="psH", bufs=2, space="PSUM") as psH,
            tc.tile_pool(name="psO", bufs=2, space="PSUM") as psO,
        ):
            xT_sb = pp.tile([128, NODES_PC], bf16)
            nc.sync.dma_start(out=xT_sb[:], in_=xT_d)
            idx_sb = pp.tile([128, S16], i16)
            nc.sync.dma_start(out=idx_sb[:], in_=idx_d)
            drel_sb = pp.tile([128, T_total], bf16)
            nc.sync.dma_start(out=drel_sb[:], in_=drel_d)
            invr_sb = pp.tile([128, NODES_PC], bf16)
            nc.sync.dma_start(out=invr_sb[:], in_=invr_d)
            invc_sb = pp.tile([128, C_PER_CORE], f32)
            nc.sync.dma_start(out=invc_sb[:], in_=invc_d)
            w1l_sb = pp.tile([D_IN, D_HID], bf16)
            nc.sync.dma_start(out=w1l_sb[:], in_=w1l_d)
            w1r_sb = pp.tile([D_IN, D_HID], bf16)
            nc.sync.dma_start(out=w1r_sb[:], in_=w1r_d)
            w2l_sb = pp.tile([D_HID, D_OUT], bf16)
            nc.sync.dma_start(out=w2l_sb[:], in_=w2l_d)
            w2r_sb = pp.tile([D_HID, D_OUT], bf16)
            nc.sync.dma_start(out=w2r_sb[:], in_=w2r_d)
            b1_sb = pp.tile([D_HID, 1], f32)
            nc.sync.dma_start(out=b1_sb[:], in_=b1_d)
            b2_sb = pp.tile([1, D_OUT], bf16)
            nc.sync.dma_start(out=b2_sb[:], in_=b2_d)
            iota_sb = pp.tile([128, 128], f32)
            nc.gpsimd.iota(
                iota_sb[:],
                pattern=[[1, 128]],
                base=0,
                channel_multiplier=0,
                allow_small_or_imprecise_dtypes=True,
            )
            iota16 = pp.tile([128, 128], bf16)
            nc.vector.tensor_copy(out=iota16[:], in_=iota_sb[:])
            iota_rep = pp.tile([128, TTmax * 128], bf16)
            for t in range(TTmax):
                nc.scalar.copy(
                    out=iota_rep[:, t * 128 : (t + 1) * 128], in_=iota16[:]
                )
            ones_sb = pp.tile([1, 128], bf16)
            nc.vector.memset(ones_sb[:], 1.0)
            h_all = pp.tile([128, NODES_PC], bf16)
            p_bounce = dp.tile([NODES_PC, 128], bf16)

            # ---------------- phase A: layer 1 + p = h @ W2_l ----------------
            tb = 0
            for j in range(n_chunks):
                tl, th, tt = TL[j], TH[j], TT[j]
                msg = mpool.tile([128, TTmax * 128], bf16, tag="msg")
                if tl:
                    nc.gpsimd.dma_gather(
                        out_ap=msg[:, : tl * 128].rearrange("p (t e) -> p t e", e=128),
                        in_ap=x_g[0:SPLIT, :],
                        idxs_ap=idx_sb[:, tb * 8 : (tb + tl) * 8],
                        num_idxs=tl * 128,
                        num_idxs_reg=tl * 128,
                        elem_size=128,
                        single_packet=False,
                    )
                if th:
                    nc.gpsimd.dma_gather(
                        out_ap=msg[:, tl * 128 : tt * 128].rearrange(
                            "p (t e) -> p t e", e=128
                        ),
                        in_ap=x_g[SPLIT:NP_, :],
                        idxs_ap=idx_sb[:, (tb + tl) * 8 : (tb + tt) * 8],
                        num_idxs=th * 128,
                        num_idxs_reg=th * 128,
                        elem_size=128,
                        single_packet=False,
                    )
                # batched one-hot build: oh[e, (t,d)] = (iota[d] == drel[e,t])
                oh = ohpool.tile([128, TTmax * 128], bf16, tag="oh")
                nc.vector.tensor_tensor(
                    out=oh[:, : tt * 128].rearrange("p (t e) -> p t e", e=128),
                    in0=iota_rep[:, : tt * 128].rearrange("p (t e) -> p t e", e=128),
                    in1=drel_sb[:, tb : tb + tt]
                    .rearrange("p (t e) -> p t e", e=1)
                    .broadcast_to([128, tt, 128]),
                    op=is_eq,
                )
                pa = psA.tile([128, 128], f32, tag="agg")
                for t in range(tt):
                    nc.tensor.matmul(
                        out=pa[:],
                        lhsT=msg[:, t * 128 : (t + 1) * 128],
                        rhs=oh[:, t * 128 : (t + 1) * 128],
                        start=(t == 0),
                        stop=(t == tt - 1),
                    )
                # meanT[f, d] = aggT * inv_deg[d]  (free-dim scale via inv_rep)
                jsl = slice(j * 128, (j + 1) * 128)
                meanT = spool.tile([128, 128], bf16, tag="meanT")
                nc.vector.tensor_tensor(
                    out=meanT[:], in0=pa[:], in1=invr_sb[:, jsl], op=mult
                )
                ph = psH.tile([128, 128], f32, tag="h")
                nc.tensor.matmul(
                    out=ph[:], lhsT=w1l_sb[:], rhs=meanT[:], start=True, stop=False
                )
                nc.tensor.matmul(
                    out=ph[:], lhsT=w1r_sb[:], rhs=xT_sb[:, jsl], start=False, stop=True
                )
                nc.scalar.activation(
                    out=h_all[:, jsl], in_=ph[:], func=relu, bias=b1_sb[:, 0:1], scale=1.0
                )
                po = psO.tile([128, D_OUT], f32, tag="p")
                nc.tensor.matmul(
                    out=po[:], lhsT=h_all[:, jsl], rhs=w2l_sb[:], start=True, stop=True
                )
                p_sb = spool.tile([128, 128], bf16, tag="p_sb")
                nc.vector.memset(p_sb[:, D_OUT:128], 0.0)
                nc.scalar.copy(out=p_sb[:, 0:D_OUT], in_=po[:])
                nc.sync.dma_start(out=p_bounce[jsl, :], in_=p_sb[:])
                tb += tt

            # ---------------- all-gather p ----------------
            nc.gpsimd.collective_compute(
                "AllGather",
                mybir.AluOpType.bypass,
                replica_groups=[list(range(N_CORES))],
                ins=[p_bounce[:]],
                outs=[p_full],
            )

            # ---------------- phase B: layer 2 ----------------
            tb = 0
            for j in range(n_chunks):
                tl, th, tt = TL[j], TH[j], TT[j]
                msg2 = mpool.tile([128, TTmax * 128], bf16, tag="msg2")
                if tl:
                    nc.gpsimd.dma_gather(
                        out_ap=msg2[:, : tl * 128].rearrange(
                            "p (t e) -> p t e", e=128
                        ),
                        in_ap=p_full[0:SPLIT, :],
                        idxs_ap=idx_sb[:, tb * 8 : (tb + tl) * 8],
                        num_idxs=tl * 128,
                        num_idxs_reg=tl * 128,
                        elem_size=128,
                        single_packet=False,
                    )
                if th:
                    nc.gpsimd.dma_gather(
                        out_ap=msg2[:, tl * 128 : tt * 128].rearrange(
                            "p (t e) -> p t e", e=128
                        ),
                        in_ap=p_full[SPLIT:NP_, :],
                        idxs_ap=idx_sb[:, (tb + tl) * 8 : (tb + tt) * 8],
                        num_idxs=th * 128,
                        num_idxs_reg=th * 128,
                        elem_size=128,
                        single_packet=False,
                    )
                jsl = slice(j * 128, (j + 1) * 128)
                oh = ohpool.tile([128, TTmax * 128], bf16, tag="oh")
                nc.vector.tensor_tensor(
                    out=oh[:, : tt * 128].rearrange("p (t e) -> p t e", e=128),
                    in0=iota_rep[:, : tt * 128].rearrange("p (t e) -> p t e", e=128),
                    in1=drel_sb[:, tb : tb + tt]
                    .rearrange("p (t e) -> p t e", e=1)
                    .broadcast_to([128, tt, 128]),
                    op=is_eq,
                )
                # agg2[d, p-feat] = sum_e oh[e, d] * msg2[e, p]
                pf = psA.tile([128, 128], f32, tag="fin")
                for t in range(tt):
                    nc.tensor.matmul(
                        out=pf[:],
                        lhsT=oh[:, t * 128 : (t + 1) * 128],
                        rhs=msg2[:, t * 128 : (t + 1) * 128],
                        start=(t == 0),
                        stop=(t == tt - 1),
                    )
                # dense part: h @ W2_r + b2  -> pd [128d, 64]
                pd = psO.tile([128, D_OUT], f32, tag="p")
                nc.tensor.matmul(
                    out=pd[:], lhsT=h_all[:, jsl], rhs=w2r_sb[:], start=True, stop=False
                )
                nc.tensor.matmul(
                    out=pd[:], lhsT=ones_sb[:], rhs=b2_sb[:], start=False, stop=True
                )
                # out = pf[:, :64] * inv_col + pd
                pd_sb = spool.tile([128, D_OUT], f32, tag="pd_sb")
                nc.scalar.copy(out=pd_sb[:], in_=pd[:])
                out_sb = spool.tile([128, D_OUT], f32, tag="out_sb")
                nc.vector.scalar_tensor_tensor(
                    out=out_sb[:],
                    in0=pf[:, 0:D_OUT],
                    scalar=invc_sb[:, j : j + 1],
                    in1=pd_sb[:],
                    op0=mult,
                    op1=add,
                )
                nc.sync.dma_start(out=out_d[jsl, :], in_=out_sb[:])
                tb += tt

    nc.compile()
    return nc


def kernel(
    x,
    edge_index,
    W1_l,
    b1,
    W1_r,
    W2_l,
    b2,
    W2_r,
):
    from concourse.bass_utils import run_bass_kernel_spmd

    per_core, TL, TH = _preprocess(x, edge_index)
    nc = _build(TL, TH)

    shared = _shared_inputs(W1_l, b1, W1_r, W2_l, b2, W2_r)
    in_maps = [{**pc, **shared} for pc in per_core]

    res = run_bass_kernel_spmd(nc, in_maps, core_ids=list(range(N_CORES)))
    out = np.concatenate([r["out"] for r in res.results], axis=0)
    return out[:N].astype(np.float32)


if __name__ == "__main__":
    rng = np.random.default_rng(0)
    x = rng.standard_normal((N, D_IN), dtype=np.float32)
    ei = rng.integers(0, N, size=(2, E), dtype=np.int64)
    s = 1.0 / np.sqrt(D_IN)
    w1l = rng.uniform(-s, s, (D_IN, D_HID)).astype(np.float32)
    w1r = rng.uniform(-s, s, (D_IN, D_HID)).astype(np.float32)
    s2 = 1.0 / np.sqrt(D_HID)
    w2l = rng.uniform(-s2, s2, (D_HID, D_OUT)).astype(np.float32)
    w2r = rng.uniform(-s2, s2, (D_HID, D_OUT)).astype(np.float32)
    out = kernel(
        x=x,
        edge_index=ei,
        W1_l=w1l,
        b1=np.zeros(D_HID, np.float32),
        W1_r=w1r,
        W2_l=w2l,
        b2=np.zeros(D_OUT, np.float32),
        W2_r=w2r,
    )
    print(out.shape, out.dtype)
